# revision 1
# baseline (speedup 1.0000x reference)
"""Trainium2 Bass kernel for a CRF layer (dense matmul potentials + Viterbi decode).

Contract: kernel(**inputs) takes the FULL unsharded inputs (numpy) and returns
(potentials, decoded_onehot), both [64, 512, 128] float32, matching reference().

Strategy (data-parallel over batch, 8 sequences per NeuronCore, SPMD on 8 cores):
  Phase 1 (TensorE): potT[v, tok] = (x @ kernel + bias + boundary)^T via matmuls
    with the bias/boundary folded in as extra contraction rows; the [tok, v]
    output layout is derived from potT by PE transpose (matmul with identity)
    instead of a second full matmul set.
  Phase 2 (forward Viterbi): replicated-slab layout. Partition p = b*16+vh owns
    the 8 next-states v = vh*8..vh*8+8 of sequence b. Each step:
      - rotation all-gather of the state vector within each 16-partition group
        via 4 stream_shuffles (order seen by partition p: u = (8*vh + j) mod 128)
      - sc = chain_perm + state_row (broadcast over vl), cmax = max over u
      - backpointers via one max_index over the flat row, decoded to absolute
        u in f32 ((idx + 8*vh - 128*vl) with one conditional -128)
      - state = where(mask, cmax + pot_t, state)
  Phase 3 (backward trace): tag_{t-1} = bp_t[tag_t] extracted with a fused
    (pi_table == tag) * bp_row -> sum  scalar_tensor_tensor per step on DVE;
    one-hot rows emitted on GpSimd and DMA-flushed to DRAM in chunks.
"""

import os
import sys

import numpy as np

sys.path.insert(0, "/opt/trn_rl_repo")

from contextlib import ExitStack

import concourse.bacc as bacc
import concourse.bass as bass
import concourse.mybir as mybir
import concourse.tile as tile

B, T, D, U = 64, 512, 1024, 128
NCORES = 8
BL = B // NCORES          # sequences per core
VH, VL = 16, 8            # partition groups of 16; 8 states per partition
TOK = BL * T              # tokens per core
KC = D // 128             # contraction chunks
TCH = 512                 # phase-1 token chunk
CH = 32                   # backward chunk (rows per bp all-gather / one-hot flush)

F32 = mybir.dt.float32
U16 = mybir.dt.uint16
U8 = mybir.dt.uint8
ALU = mybir.AluOpType
AX = mybir.AxisListType
NEGBIG = -3.0e38

def _shuffle_mask(s):
    # quadrant-local: keep the 16-group bit, rotate within the group by s
    return [(i & 16) | ((i + s) & 15) for i in range(32)]


def build_module(t_steps=T, tok=TOK, n_devices=NCORES, pool_argmax=8):
    """Build the SPMD Bass module. t_steps/tok shrinkable for simulation.

    pool_argmax: how many of the 8 vl columns extract backpointers on GpSimd
    via (sc == cmax) * pi -> sum; the rest use DVE max_index + index decode.
    """
    nc = bacc.Bacc(
        "TRN2", target_bir_lowering=False, debug=False, num_devices=n_devices
    )

    tch = min(TCH, tok)
    n_tch = tok // tch
    ch = min(CH, t_steps)
    n_ch = t_steps // ch
    assert t_steps % ch == 0 and tok % tch == 0 and tch % 128 == 0

    # ---- DRAM I/O ----
    x_tokT = nc.dram_tensor("x_tokT", [D, tok], F32, kind="ExternalInput")
    w_sb_h = nc.dram_tensor("w_chunks", [128, KC * 128], F32, kind="ExternalInput")
    aug_uv = nc.dram_tensor("aug_uv", [3, 128], F32, kind="ExternalInput")
    aug_tok = nc.dram_tensor("aug_tok", [3, tok], F32, kind="ExternalInput")
    chain_perm = nc.dram_tensor("chain_perm", [128, VL * 128], F32, kind="ExternalInput")
    pi_tab = nc.dram_tensor("pi_tab", [128, 128], F32, kind="ExternalInput")
    u_iota = nc.dram_tensor("u_iota", [128, 128], F32, kind="ExternalInput")
    ident = nc.dram_tensor("ident", [128, 128], F32, kind="ExternalInput")
    v_iota = nc.dram_tensor("v_iota", [128, VL], F32, kind="ExternalInput")
    bpc1 = nc.dram_tensor("bpc1", [128, VL], U16, kind="ExternalInput")
    cfdec = nc.dram_tensor("cfdec", [128, VL], F32, kind="ExternalInput")
    vh8u = nc.dram_tensor("vh8u", [128, 1], U16, kind="ExternalInput")
    mask_rep = nc.dram_tensor("mask_rep", [128, t_steps], U8, kind="ExternalInput")

    out_pot = nc.dram_tensor("out_pot", [tok, U], F32, kind="ExternalOutput")
    out_oh = nc.dram_tensor("out_oh", [tok, U], F32, kind="ExternalOutput")
    potT_dram = nc.dram_tensor("potT_scratch", [U, tok], F32, kind="Internal")

    with tile.TileContext(nc) as tc, ExitStack() as ctx:
        persist = ctx.enter_context(tc.tile_pool(name="persist", bufs=1))

        # ---- persistent SBUF tiles ----
        w_sb = persist.tile([128, KC * 128], F32, tag="w_sb")
        aug_uv_sb = persist.tile([3, 128], F32, tag="aug_uv")
        aug_tok_sb = persist.tile([3, tok], F32, tag="aug_tok")
        chain_sb = persist.tile([128, VL, 128], F32, tag="chain")
        pi_sb = persist.tile([128, 128], F32, tag="pi")
        uio_sb = persist.tile([128, 128], F32, tag="uio")
        ident_sb = persist.tile([128, 128], F32, tag="ident")
        vio_sb = persist.tile([128, VL], F32, tag="vio")
        bpc1_sb = persist.tile([128, VL], U16, tag="bpc1")
        cfdec_sb = persist.tile([128, VL], F32, tag="cfdec")
        vh8u_sb = persist.tile([128, 1], U16, tag="vh8u")
        mask_sb = persist.tile([128, t_steps], U8, tag="mask")
        potT_sb = persist.tile([128, tok], F32, tag="potT")
        pot_rep = persist.tile([128, VL, t_steps], F32, tag="pot_rep")
        state_row = persist.tile([128, 128], F32, tag="state_row")
        bp_store = persist.tile([128, t_steps, VL], F32, tag="bp_store")

        nc.sync.dma_start(w_sb[:], w_sb_h.ap())
        nc.sync.dma_start(aug_uv_sb[:], aug_uv.ap())
        nc.sync.dma_start(aug_tok_sb[:], aug_tok.ap())
        nc.sync.dma_start(chain_sb[:].rearrange("p a b -> p (a b)"), chain_perm.ap())
        nc.sync.dma_start(pi_sb[:], pi_tab.ap())
        nc.sync.dma_start(uio_sb[:], u_iota.ap())
        nc.sync.dma_start(ident_sb[:], ident.ap())
        nc.sync.dma_start(vio_sb[:], v_iota.ap())
        nc.sync.dma_start(bpc1_sb[:], bpc1.ap())
        nc.sync.dma_start(cfdec_sb[:], cfdec.ap())
        nc.sync.dma_start(vh8u_sb[:], vh8u.ap())
        nc.sync.dma_start(mask_sb[:], mask_rep.ap())

        # ================= Phase 1: potentials matmuls =================
        with tc.tile_pool(name="ph1", bufs=2) as ph1, \
             tc.tile_pool(name="psA", bufs=2, space="PSUM") as psA_pool, \
             tc.tile_pool(name="psB", bufs=2, space="PSUM") as psB_pool:
            for tc_i in range(n_tch):
                t0 = tc_i * tch
                xT = ph1.tile([128, KC, tch], F32, tag="xT")
                for k in range(KC):
                    nc.sync.dma_start(
                        xT[:, k, :],
                        bass.AP(x_tokT, k * 128 * tok + t0, [[tok, 128], [1, tch]]),
                    )
                # potT[v, tok] += sum_k w[k]^T x[k]  (+ bias/boundary rows)
                psA = psA_pool.tile([128, tch], F32, tag="psA")
                for k in range(KC):
                    nc.tensor.matmul(
                        psA[:], w_sb[:, k * 128:(k + 1) * 128], xT[:, k, :],
                        start=(k == 0), stop=False,
                    )
                nc.tensor.matmul(
                    psA[:], aug_uv_sb[:], aug_tok_sb[:, t0:t0 + tch],
                    start=False, stop=True,
                )
                nc.vector.tensor_copy(potT_sb[:, t0:t0 + tch], psA[:])
                # pot[tok, v]: transpose potT slabs on the PE (identity matmul)
                for s in range(tch // 128):
                    s0 = t0 + s * 128
                    psB = psB_pool.tile([128, 128], F32, tag="psB")
                    nc.tensor.matmul(
                        psB[:], potT_sb[:, s0:s0 + 128], ident_sb[:],
                        start=True, stop=True,
                    )
                    po = ph1.tile([128, 128], F32, tag="po")
                    nc.vector.tensor_copy(po[:], psB[:])
                    nc.sync.dma_start(
                        bass.AP(out_pot, s0 * U, [[U, 128], [1, U]]), po[:]
                    )

        # potT -> DRAM -> pot_rep[(b,vh), vl, t] = pot[b, t, 8*vh+vl]
        nc.sync.dma_start(potT_dram.ap(), potT_sb[:])
        for b in range(BL):
            nc.sync.dma_start(
                pot_rep[b * VH:(b + 1) * VH, :, :],
                bass.AP(potT_dram, b * t_steps,
                        [[VL * tok, VH], [tok, VL], [1, t_steps]]),
            )

        # ================= Phase 2: forward Viterbi =================
        # bp init = identity (padding rows keep it); state init = potentials[:, 0]
        nc.vector.tensor_copy(
            bp_store[:],
            vio_sb[:].unsqueeze(1).broadcast_to([128, t_steps, VL]),
        )
        nc.vector.tensor_copy(state_row[:, 0:VL], pot_rep[:, :, 0])

        masks = {s: _shuffle_mask(s) for s in (1, 2, 4, 8)}
        fwd = ctx.enter_context(tc.tile_pool(name="fwd", bufs=3))
        for t in range(1, t_steps):
            for s in (1, 2, 4, 8):
                nc.vector.stream_shuffle(
                    state_row[:, 8 * s:16 * s], state_row[:, 0:8 * s], masks[s]
                )
            sc = fwd.tile([128, VL, 128], F32, tag="sc")
            nc.vector.tensor_tensor(
                sc[:], chain_sb[:],
                state_row[:].unsqueeze(1).broadcast_to([128, VL, 128]),
                ALU.add,
            )
            cmax = fwd.tile([128, VL], F32, tag="cmax")
            nc.vector.tensor_reduce(cmax[:], sc[:], AX.X, ALU.max)
            idx = fwd.tile([128, VL], U16, tag="idx")
            nc.vector.max_index(idx[:], cmax[:], sc[:].rearrange("p a b -> p (a b)"))
            # state update (masked)
            mbc = mask_sb[:, t:t + 1].broadcast_to([128, VL])
            tmp = fwd.tile([128, VL], F32, tag="tmp")
            nc.vector.tensor_tensor(tmp[:], cmax[:], pot_rep[:, :, t], ALU.add)
            nc.vector.copy_predicated(state_row[:, 0:VL], mbc, tmp[:])
            # bp decode on GpSimd (f32): u = (idx + 8*vh - 128*vl) mod 128,
            # where the pre-mod value is < 256 so one conditional -128 works.
            bp0 = fwd.tile([128, VL], F32, tag="bp0")
            nc.vector.tensor_copy(bp0[:], idx[:])
            bp1 = fwd.tile([128, VL], F32, tag="bp1")
            nc.vector.tensor_tensor(bp1[:], bp0[:], cfdec_sb[:], ALU.add)
            ge = fwd.tile([128, VL], F32, tag="ge")
            nc.vector.tensor_scalar(ge[:], bp1[:], 128.0, None, ALU.is_ge)
            gm = fwd.tile([128, VL], F32, tag="gm")
            nc.vector.tensor_scalar(gm[:], ge[:], -128.0, None, ALU.mult)
            bpf = fwd.tile([128, VL], F32, tag="bpf")
            nc.vector.tensor_tensor(bpf[:], bp1[:], gm[:], ALU.add)
            nc.vector.copy_predicated(bp_store[:, t, :], mbc, bpf[:])

        # ================= Phase 3: backward trace + one-hot =================
        for s in (1, 2, 4, 8):
            nc.vector.stream_shuffle(
                state_row[:, 8 * s:16 * s], state_row[:, 0:8 * s], masks[s]
            )
        bwd = ctx.enter_context(tc.tile_pool(name="bwd", bufs=2))
        top8 = bwd.tile([128, 8], F32, tag="top8")
        nc.vector.max(top8[:], state_row[:])
        idx8 = bwd.tile([128, 8], U16, tag="idx8")
        nc.vector.max_index(idx8[:], top8[:], state_row[:])
        lt1 = bwd.tile([128, 1], U16, tag="lt1")
        nc.vector.tensor_tensor(lt1[:], idx8[:, 0:1], vh8u_sb[:], ALU.add)
        lt2 = bwd.tile([128, 1], U16, tag="lt2")
        nc.vector.tensor_scalar(lt2[:], lt1[:], 127, None, ALU.bitwise_and)
        tag_prev = bwd.tile([128, 1], F32, tag="tag")
        nc.vector.tensor_copy(tag_prev[:], lt2[:])

        ring_pool = ctx.enter_context(tc.tile_pool(name="ring", bufs=2))
        oh_pool = ctx.enter_context(tc.tile_pool(name="ohr", bufs=2))
        sel_pool = ctx.enter_context(tc.tile_pool(name="sel", bufs=3))

        oh_ring = oh_pool.tile([128, ch, 128], F32, tag="ohring")
        nc.gpsimd.tensor_scalar(
            oh_ring[:, ch - 1, :], uio_sb[:], tag_prev[:, 0:1], None, ALU.is_equal
        )

        for c in range(n_ch - 1, -1, -1):
            tbase = c * ch
            ring = ring_pool.tile([128, ch, 128], F32, tag="bpring")
            nc.vector.tensor_copy(ring[:, :, 0:VL], bp_store[:, tbase:tbase + ch, :])
            for s in (1, 2, 4, 8):
                nc.vector.stream_shuffle(
                    ring[:, :, 8 * s:16 * s], ring[:, :, 0:8 * s], masks[s]
                )
            t_lo = max(tbase, 1)
            for t in range(tbase + ch - 1, t_lo - 1, -1):
                sel = sel_pool.tile([128, 128], F32, tag="sel")
                tag_new = sel_pool.tile([128, 1], F32, tag="tagn")
                nc.vector.scalar_tensor_tensor(
                    sel[:], pi_sb[:], tag_prev[:, 0:1], ring[:, t - tbase, :],
                    ALU.is_equal, ALU.mult, accum_out=tag_new[:],
                )
                # one-hot row for tag_{t-1} (GpSimd, off the DVE trace chain)
                r = t - 1
                if r % ch == ch - 1:
                    oh_ring = oh_pool.tile([128, ch, 128], F32, tag="ohring")
                nc.gpsimd.tensor_scalar(
                    oh_ring[:, r % ch, :], uio_sb[:], tag_new[:, 0:1], None,
                    ALU.is_equal,
                )
                tag_prev = tag_new
                if r % ch == 0:
                    nc.sync.dma_start(
                        bass.AP(out_oh, r * U, [[t_steps * U, BL], [1, ch * U]]),
                        oh_ring[0:128:VH, :, :].rearrange("p t v -> p (t v)"),
                    )

    nc.compile()
    if not nc.is_finalized():
        nc.finalize()
    return nc


def _host_prep(inputs, mask, kern, bias, chain_kernel, left_b, right_b, t_steps=T):
    """Build per-core input maps (all numpy, float32)."""
    tok = BL * t_steps
    p = np.arange(128)
    vh = p % VH
    j = np.arange(128)
    # pi[p, j] = (8*vh + j) mod 128 : state/bp all-gather order per partition
    pi = (8 * vh[:, None] + j[None, :]) % 128
    v_of_p = vh[:, None] * VL + np.arange(VL)[None, :]  # [128, VL]

    chain_pp = np.empty((128, VL, 128), np.float32)
    for pp in range(128):
        chain_pp[pp] = chain_kernel[pi[pp]][:, v_of_p[pp]].T  # [VL, 128]

    w_chunks = kern.reshape(KC, 128, 128).transpose(1, 0, 2).reshape(128, KC * 128)
    aug_uv = np.stack([bias, left_b, right_b]).astype(np.float32)

    lengths = mask.sum(axis=1).astype(np.int64)
    n_cores = inputs.shape[0] // BL
    in_maps = []
    for c in range(n_cores):
        bs = c * BL
        xl = inputs[bs:bs + BL].reshape(tok, D)
        ones = np.ones(tok, np.float32)
        start01 = np.zeros((BL, t_steps), np.float32)
        end01 = np.zeros((BL, t_steps), np.float32)
        start01[:, 0] = 1.0
        for b in range(BL):
            end01[b, lengths[bs + b] - 1] = 1.0
        m = {
            "x_tokT": np.ascontiguousarray(xl.T),
            "w_chunks": np.ascontiguousarray(w_chunks),
            "aug_uv": np.ascontiguousarray(aug_uv),
            "aug_tok": np.ascontiguousarray(
                np.stack([ones, start01.ravel(), end01.ravel()])),
            "chain_perm": np.ascontiguousarray(chain_pp.reshape(128, VL * 128)),
            "pi_tab": pi.astype(np.float32),
            "u_iota": np.tile(j.astype(np.float32), (128, 1)),
            "ident": np.eye(128, dtype=np.float32),
            "v_iota": v_of_p.astype(np.float32),
            "bpc1": np.tile((128 * np.arange(VL, dtype=np.uint16))[None, :],
                            (128, 1)),
            "cfdec": (8.0 * vh[:, None] - 128.0 * np.arange(VL)[None, :]
                      ).astype(np.float32),
            "vh8u": (8 * vh[:, None]).astype(np.uint16),
            "mask_rep": mask[bs + p // VH].astype(np.uint8),
        }
        in_maps.append(m)
    return in_maps


_NC_CACHE = {}


def kernel(inputs, mask, kernel, bias, chain_kernel, left_boundary, right_boundary):
    inputs = np.asarray(inputs, np.float32)
    mask_np = np.asarray(mask)
    kern = np.asarray(kernel, np.float32)
    bias = np.asarray(bias, np.float32)
    chain = np.asarray(chain_kernel, np.float32)
    lb = np.asarray(left_boundary, np.float32)
    rb = np.asarray(right_boundary, np.float32)

    from concourse.bass_utils import run_bass_kernel_spmd

    if "nc" not in _NC_CACHE:
        _NC_CACHE["nc"] = build_module()
    nc = _NC_CACHE["nc"]

    in_maps = _host_prep(inputs, mask_np, kern, bias, chain, lb, rb)
    res = run_bass_kernel_spmd(
        nc, in_maps, core_ids=list(range(NCORES)),
        trace=bool(int(os.environ.get("KERNEL_TRACE", "0"))),
    )
    pot = np.concatenate(
        [r["out_pot"].reshape(BL, T, U) for r in res.results], axis=0)
    oh = np.concatenate(
        [r["out_oh"].reshape(BL, T, U) for r in res.results], axis=0)
    if res.exec_time_ns is not None:
        print(f"HW exec time: {res.exec_time_ns} ns")
    return pot, oh



# revision 4
# speedup vs baseline: 1.2495x; 1.2495x over previous
"""Trainium2 Bass kernel for a CRF layer (dense matmul potentials + Viterbi decode).

Contract: kernel(**inputs) takes the FULL unsharded inputs (numpy) and returns
(potentials, decoded_onehot), both [64, 512, 128] float32, matching reference().

Strategy (data-parallel over batch, 8 sequences per NeuronCore, SPMD on 8 cores):
  Phase 1 (TensorE): potT[v, tok] = (x @ kernel + bias + boundary)^T via matmuls
    with the bias/boundary folded in as extra contraction rows. potT goes to
    DRAM as-is; the host transposes to [tok, v] (pure layout move).
  Phase 2 (forward Viterbi): replicated-slab layout. Partition p = b*16+vh owns
    the 8 next-states v = vh*8..vh*8+8 of sequence b. Per step:
      - rotation all-gather of the state vector within each 16-partition group
        via 4 stream_shuffles (order seen by partition p: u = (8*vh + j) mod 128)
      - sc = chain_perm + state_row broadcast (optionally split DVE/GpSimd)
      - cmax = max over u (tensor_reduce); backpointers = raw max_index (u16)
        written straight into bp_raw, deferred one step so it overlaps the
        next step's adds; no per-step decode, no bp masking.
      - state = where(mask, cmax + pot_t, state)
  Phase 2b: batched decode of raw indices -> absolute predecessor states,
    done per 64-step chunk right before the backward pass consumes them.
  Phase 3 (backward trace): per step one is_eq (pi == tag) + one
    tensor_tensor_reduce (sel*bp -> sum) + copy_predicated(~mask) to freeze
    tags across padding. Tags land in tag_store[128, T].
  Phase 3b: tag_store -> PE transpose -> 32 bulk is_eq one-hot tiles -> DMA.
"""

import os
import sys

import numpy as np

sys.path.insert(0, "/opt/trn_rl_repo")

from contextlib import ExitStack

import concourse.bacc as bacc
import concourse.bass as bass
import concourse.mybir as mybir
import concourse.tile as tile

B, T, D, U = 64, 512, 1024, 128
NCORES = 8
BL = B // NCORES          # sequences per core
VH, VL = 16, 8            # partition groups of 16; 8 states per partition
TOK = BL * T              # tokens per core
KC = D // 128             # contraction chunks
TCH = 512                 # phase-1 token chunk
CH = 64                   # backward chunk (rows per bp decode/all-gather)

F32 = mybir.dt.float32
U16 = mybir.dt.uint16
U8 = mybir.dt.uint8
ALU = mybir.AluOpType
AX = mybir.AxisListType


def _shuffle_mask(s):
    # quadrant-local: keep the 16-group bit, rotate within the group by s
    return [(i & 16) | ((i + s) & 15) for i in range(32)]


def build_module(t_steps=T, tok=TOK, n_devices=NCORES, add_split=0):
    """Build the SPMD Bass module. t_steps/tok shrinkable for simulation.

    add_split: vl groups [add_split..8) of the per-step sc add run on GpSimd
    (0 = everything on DVE).
    """
    nc = bacc.Bacc(
        "TRN2", target_bir_lowering=False, debug=False, num_devices=n_devices
    )

    tch = min(TCH, tok)
    n_tch = tok // tch
    ch = min(CH, t_steps)
    n_ch = t_steps // ch
    assert t_steps % ch == 0 and tok % tch == 0 and tch % 128 == 0

    # ---- DRAM I/O ----
    x_tokT = nc.dram_tensor("x_tokT", [D, tok], F32, kind="ExternalInput")
    w_sb_h = nc.dram_tensor("w_chunks", [128, KC * 128], F32, kind="ExternalInput")
    aug_uv = nc.dram_tensor("aug_uv", [3, 128], F32, kind="ExternalInput")
    aug_tok = nc.dram_tensor("aug_tok", [3, tok], F32, kind="ExternalInput")
    chain_perm = nc.dram_tensor("chain_perm", [128, VL * 128], F32, kind="ExternalInput")
    pi_tab = nc.dram_tensor("pi_tab", [128, 128], F32, kind="ExternalInput")
    u_iota = nc.dram_tensor("u_iota", [128, 128], F32, kind="ExternalInput")
    ident = nc.dram_tensor("ident", [128, 128], F32, kind="ExternalInput")
    cfdec = nc.dram_tensor("cfdec", [128, VL], F32, kind="ExternalInput")
    vh8u = nc.dram_tensor("vh8u", [128, 1], U16, kind="ExternalInput")
    mask_rep = nc.dram_tensor("mask_rep", [128, t_steps], U8, kind="ExternalInput")
    nmask_rep = nc.dram_tensor("nmask_rep", [128, t_steps], U8, kind="ExternalInput")

    out_potT = nc.dram_tensor("out_potT", [U, tok], F32, kind="ExternalOutput")
    out_oh = nc.dram_tensor("out_oh", [tok, U], F32, kind="ExternalOutput")

    with tile.TileContext(nc) as tc, ExitStack() as ctx:
        persist = ctx.enter_context(tc.tile_pool(name="persist", bufs=1))

        # ---- persistent SBUF tiles ----
        w_sb = persist.tile([128, KC * 128], F32, tag="w_sb")
        aug_uv_sb = persist.tile([3, 128], F32, tag="aug_uv")
        aug_tok_sb = persist.tile([3, tok], F32, tag="aug_tok")
        chain_sb = persist.tile([128, VL, 128], F32, tag="chain")
        pi_sb = persist.tile([128, 128], F32, tag="pi")
        uio_sb = persist.tile([128, 128], F32, tag="uio")
        ident_sb = persist.tile([128, 128], F32, tag="ident")
        cfdec_sb = persist.tile([128, VL], F32, tag="cfdec")
        vh8u_sb = persist.tile([128, 1], U16, tag="vh8u")
        mask_sb = persist.tile([128, t_steps], U8, tag="mask")
        nmask_sb = persist.tile([128, t_steps], U8, tag="nmask")
        pot_rep = persist.tile([128, VL, t_steps], F32, tag="pot_rep")
        state_row = persist.tile([128, 128], F32, tag="state_row")
        bp_raw = persist.tile([128, t_steps, VL], U16, tag="bp_raw")
        tag_store = persist.tile([128, t_steps], F32, tag="tag_store")

        nc.sync.dma_start(w_sb[:], w_sb_h.ap())
        nc.sync.dma_start(aug_uv_sb[:], aug_uv.ap())
        nc.sync.dma_start(aug_tok_sb[:], aug_tok.ap())
        nc.sync.dma_start(chain_sb[:].rearrange("p a b -> p (a b)"), chain_perm.ap())
        nc.sync.dma_start(pi_sb[:], pi_tab.ap())
        nc.sync.dma_start(uio_sb[:], u_iota.ap())
        nc.sync.dma_start(ident_sb[:], ident.ap())
        nc.sync.dma_start(cfdec_sb[:], cfdec.ap())
        nc.sync.dma_start(vh8u_sb[:], vh8u.ap())
        nc.sync.dma_start(mask_sb[:], mask_rep.ap())
        nc.sync.dma_start(nmask_sb[:], nmask_rep.ap())

        # ================= Phase 1: potentials matmuls =================
        with tc.tile_pool(name="ph1", bufs=2) as ph1, \
             tc.tile_pool(name="ph1o", bufs=2) as ph1o, \
             tc.tile_pool(name="psA", bufs=2, space="PSUM") as psA_pool:
            for tc_i in range(n_tch):
                t0 = tc_i * tch
                xT = ph1.tile([128, KC, tch], F32, tag="xT")
                for k in range(KC):
                    nc.sync.dma_start(
                        xT[:, k, :],
                        bass.AP(x_tokT, k * 128 * tok + t0, [[tok, 128], [1, tch]]),
                    )
                # potT[v, tok] += sum_k w[k]^T x[k]  (+ bias/boundary rows)
                psA = psA_pool.tile([128, tch], F32, tag="psA")
                for k in range(KC):
                    nc.tensor.matmul(
                        psA[:], w_sb[:, k * 128:(k + 1) * 128], xT[:, k, :],
                        start=(k == 0), stop=False,
                    )
                nc.tensor.matmul(
                    psA[:], aug_uv_sb[:], aug_tok_sb[:, t0:t0 + tch],
                    start=False, stop=True,
                )
                po = ph1o.tile([128, tch], F32, tag="po")
                nc.vector.tensor_copy(po[:], psA[:])
                nc.sync.dma_start(
                    bass.AP(out_potT, t0, [[tok, 128], [1, tch]]), po[:]
                )

        # potT (DRAM) -> pot_rep[(b,vh), vl, t] = pot[b, t, 8*vh+vl]
        for b in range(BL):
            nc.sync.dma_start(
                pot_rep[b * VH:(b + 1) * VH, :, :],
                bass.AP(out_potT, b * t_steps,
                        [[VL * tok, VH], [tok, VL], [1, t_steps]]),
            )

        # ================= Phase 2: forward Viterbi =================
        nc.vector.tensor_copy(state_row[:, 0:VL], pot_rep[:, :, 0])
        nc.vector.memset(bp_raw[:, 0, :], 0)

        masks = {s: _shuffle_mask(s) for s in (1, 2, 4, 8)}
        sc_pool = ctx.enter_context(tc.tile_pool(name="sc", bufs=2))
        fwd = ctx.enter_context(tc.tile_pool(name="fwd", bufs=2))
        prev = None  # (sc, cmax, t) awaiting backpointer extraction
        for t in range(1, t_steps):
            for s in (1, 2, 4, 8):
                nc.vector.stream_shuffle(
                    state_row[:, 8 * s:16 * s], state_row[:, 0:8 * s], masks[s]
                )
            sc = sc_pool.tile([128, VL, 128], F32, tag="sc")
            srow_b = state_row[:].unsqueeze(1)
            if add_split > 0:
                nc.gpsimd.tensor_tensor(
                    sc[:, add_split:, :], chain_sb[:, add_split:, :],
                    srow_b.broadcast_to([128, VL - add_split, 128]), ALU.add,
                )
                nc.vector.tensor_tensor(
                    sc[:, 0:add_split, :], chain_sb[:, 0:add_split, :],
                    srow_b.broadcast_to([128, add_split, 128]), ALU.add,
                )
            else:
                nc.vector.tensor_tensor(
                    sc[:], chain_sb[:], srow_b.broadcast_to([128, VL, 128]), ALU.add
                )
            # deferred backpointer extraction for the previous step (keeps the
            # DVE busy while GpSimd/DVE finish this step's adds)
            if prev is not None:
                psc, pcm, pt = prev
                nc.vector.max_index(
                    bp_raw[:, pt, :], pcm[:],
                    psc[:].rearrange("p a b -> p (a b)"),
                )
            cmax = fwd.tile([128, VL], F32, tag="cmax")
            nc.vector.tensor_reduce(cmax[:], sc[:], AX.X, ALU.max)
            tmp = fwd.tile([128, VL], F32, tag="tmp")
            nc.vector.tensor_tensor(tmp[:], cmax[:], pot_rep[:, :, t], ALU.add)
            nc.vector.copy_predicated(
                state_row[:, 0:VL],
                mask_sb[:, t:t + 1].broadcast_to([128, VL]), tmp[:],
            )
            prev = (sc, cmax, t)
        psc, pcm, pt = prev
        nc.vector.max_index(
            bp_raw[:, pt, :], pcm[:],
            psc[:].rearrange("p a b -> p (a b)"),
        )

        # ---- final tag: argmax of the (mask-frozen) final state ----
        bwd = ctx.enter_context(tc.tile_pool(name="bwd", bufs=1))
        for s in (1, 2, 4, 8):
            nc.vector.stream_shuffle(
                state_row[:, 8 * s:16 * s], state_row[:, 0:8 * s], masks[s]
            )
        top8 = bwd.tile([128, 8], F32, tag="top8")
        nc.vector.max(top8[:], state_row[:])
        idx8 = bwd.tile([128, 8], U16, tag="idx8")
        nc.vector.max_index(idx8[:], top8[:], state_row[:])
        lt1 = bwd.tile([128, 1], U16, tag="lt1")
        nc.vector.tensor_tensor(lt1[:], idx8[:, 0:1], vh8u_sb[:], ALU.add)
        lt2 = bwd.tile([128, 1], U16, tag="lt2")
        nc.vector.tensor_scalar(lt2[:], lt1[:], 127, None, ALU.bitwise_and)
        nc.vector.tensor_copy(tag_store[:, t_steps - 1:t_steps], lt2[:])

        # ================= Phase 3: backward trace =================
        ring_pool = ctx.enter_context(tc.tile_pool(name="ring", bufs=2))
        dec_pool = ctx.enter_context(tc.tile_pool(name="dec", bufs=2))
        sel_pool = ctx.enter_context(tc.tile_pool(name="sel", bufs=2))

        for c in range(n_ch - 1, -1, -1):
            tb = c * ch
            # decode raw indices for this chunk: u = (idx + 8*vh - 128*vl),
            # minus 128 when >= 128 (value known < 256)
            scrA = dec_pool.tile([128, ch, VL], F32, tag="scrA")
            scrB = dec_pool.tile([128, ch, VL], F32, tag="scrB")
            nc.vector.tensor_copy(scrA[:], bp_raw[:, tb:tb + ch, :])
            nc.vector.tensor_tensor(
                scrB[:], scrA[:],
                cfdec_sb[:].unsqueeze(1).broadcast_to([128, ch, VL]), ALU.add,
            )
            nc.vector.tensor_scalar(scrA[:], scrB[:], 128.0, None, ALU.is_ge)
            ring = ring_pool.tile([128, ch, 128], F32, tag="bpring")
            nc.vector.scalar_tensor_tensor(
                ring[:, :, 0:VL], scrA[:], -128.0, scrB[:], ALU.mult, ALU.add
            )
            for s in (1, 2, 4, 8):
                nc.vector.stream_shuffle(
                    ring[:, :, 8 * s:16 * s], ring[:, :, 0:8 * s], masks[s]
                )
            t_lo = max(tb, 1)
            for t in range(tb + ch - 1, t_lo - 1, -1):
                sel = sel_pool.tile([128, 128], F32, tag="sel")
                nc.vector.scalar_tensor_tensor(
                    sel[:], pi_sb[:], tag_store[:, t:t + 1], ring[:, t - tb, :],
                    ALU.is_equal, ALU.mult, accum_out=tag_store[:, t - 1:t],
                )
                nc.vector.copy_predicated(
                    tag_store[:, t - 1:t], nmask_sb[:, t:t + 1],
                    tag_store[:, t:t + 1],
                )

        # ================= Phase 3b: bulk one-hot =================
        with tc.tile_pool(name="oh", bufs=3) as oh_pool, \
             tc.tile_pool(name="tt", bufs=1) as tt_pool, \
             tc.tile_pool(name="psT", bufs=2, space="PSUM") as psT_pool:
            n_tc = (t_steps + 127) // 128
            tagT = tt_pool.tile([128, n_tc, VL], F32, tag="tagT")
            for cc in range(n_tc):
                c0 = cc * 128
                clen = min(128, t_steps - c0)
                psT = psT_pool.tile([clen, 128], F32, tag="psT")
                nc.tensor.matmul(
                    psT[:], tag_store[:, c0:c0 + clen], ident_sb[:],
                    start=True, stop=True,
                )
                nc.vector.tensor_copy(
                    tagT[0:clen, cc, :], psT[:, 0:128:VH]
                )
            for b in range(BL):
                for cc in range(n_tc):
                    c0 = cc * 128
                    clen = min(128, t_steps - c0)
                    oh_t = oh_pool.tile([128, 128], F32, tag="oh")
                    nc.vector.tensor_tensor(
                        oh_t[0:clen, :], uio_sb[0:clen, :],
                        tagT[0:clen, cc, b:b + 1].broadcast_to([clen, 128]),
                        ALU.is_equal,
                    )
                    nc.sync.dma_start(
                        bass.AP(out_oh, (b * t_steps + c0) * U, [[U, clen], [1, U]]),
                        oh_t[0:clen, :],
                    )

    nc.compile()
    if not nc.is_finalized():
        nc.finalize()
    return nc


def _host_prep(inputs, mask, kern, bias, chain_kernel, left_b, right_b, t_steps=T):
    """Build per-core input maps (all numpy, float32)."""
    tok = BL * t_steps
    p = np.arange(128)
    vh = p % VH
    j = np.arange(128)
    # pi[p, j] = (8*vh + j) mod 128 : state/bp all-gather order per partition
    pi = (8 * vh[:, None] + j[None, :]) % 128
    v_of_p = vh[:, None] * VL + np.arange(VL)[None, :]  # [128, VL]

    chain_pp = np.empty((128, VL, 128), np.float32)
    for pp in range(128):
        chain_pp[pp] = chain_kernel[pi[pp]][:, v_of_p[pp]].T  # [VL, 128]

    w_chunks = kern.reshape(KC, 128, 128).transpose(1, 0, 2).reshape(128, KC * 128)
    aug_uv = np.stack([bias, left_b, right_b]).astype(np.float32)

    lengths = mask.sum(axis=1).astype(np.int64)
    n_cores = inputs.shape[0] // BL
    in_maps = []
    for c in range(n_cores):
        bs = c * BL
        xl = inputs[bs:bs + BL].reshape(tok, D)
        ones = np.ones(tok, np.float32)
        start01 = np.zeros((BL, t_steps), np.float32)
        end01 = np.zeros((BL, t_steps), np.float32)
        start01[:, 0] = 1.0
        for b in range(BL):
            end01[b, lengths[bs + b] - 1] = 1.0
        m = {
            "x_tokT": np.ascontiguousarray(xl.T),
            "w_chunks": np.ascontiguousarray(w_chunks),
            "aug_uv": np.ascontiguousarray(aug_uv),
            "aug_tok": np.ascontiguousarray(
                np.stack([ones, start01.ravel(), end01.ravel()])),
            "chain_perm": np.ascontiguousarray(chain_pp.reshape(128, VL * 128)),
            "pi_tab": pi.astype(np.float32),
            "u_iota": np.tile(j.astype(np.float32), (128, 1)),
            "ident": np.eye(128, dtype=np.float32),
            "cfdec": (8.0 * vh[:, None] - 128.0 * np.arange(VL)[None, :]
                      ).astype(np.float32),
            "vh8u": (8 * vh[:, None]).astype(np.uint16),
            "mask_rep": mask[bs + p // VH, :t_steps].astype(np.uint8),
            "nmask_rep": (~mask[bs + p // VH, :t_steps]).astype(np.uint8),
        }
        in_maps.append(m)
    return in_maps


_NC_CACHE = {}


def kernel(inputs, mask, kernel, bias, chain_kernel, left_boundary, right_boundary):
    inputs = np.asarray(inputs, np.float32)
    mask_np = np.asarray(mask)
    kern = np.asarray(kernel, np.float32)
    bias = np.asarray(bias, np.float32)
    chain = np.asarray(chain_kernel, np.float32)
    lb = np.asarray(left_boundary, np.float32)
    rb = np.asarray(right_boundary, np.float32)

    from concourse.bass_utils import run_bass_kernel_spmd

    add_split = int(os.environ.get("KERNEL_ADD_SPLIT", "0"))
    key = ("nc", add_split)
    if key not in _NC_CACHE:
        _NC_CACHE[key] = build_module(add_split=add_split)
    nc = _NC_CACHE[key]

    in_maps = _host_prep(inputs, mask_np, kern, bias, chain, lb, rb)
    res = run_bass_kernel_spmd(
        nc, in_maps, core_ids=list(range(NCORES)),
        trace=bool(int(os.environ.get("KERNEL_TRACE", "0"))),
    )
    pot = np.concatenate(
        [np.ascontiguousarray(r["out_potT"].reshape(U, BL, T).transpose(1, 2, 0))
         for r in res.results], axis=0)
    oh = np.concatenate(
        [r["out_oh"].reshape(BL, T, U) for r in res.results], axis=0)
    if res.exec_time_ns is not None:
        print(f"HW exec time: {res.exec_time_ns} ns")
    return pot, oh
